# Initial kernel scaffold
#
"""Fused causal-attention block (QKV proj + attention + out proj) for
nn_Attn_49881750176227 on 8 TRN2 NeuronCores.

Sharding (hardcoded): core c -> batch b = c // 2, head-group hg = c % 2.
Each core handles 1 batch (2048 tokens) and 8 of the 16 heads, with the
matching column/row slices of W1 / out_proj (tensor parallel).  The
out-projection partial sums of the two cores sharing a batch are added
on the host during unsharding; k/v cache outputs shard cleanly by head.

Device layout (per core):
  x_t   [1024, 2048]  X[b].T                 (in)
  wqk_t [1024, 1024]  [Wq_shard; Wk_shard].T (in)
  bqk   [128, 8]      qk bias, tile-major    (in)
  wv_t  [1024, 512]   Wv_shard.T             (in)
  bv    [1, 512]      v bias row             (in)
  ow_t  [512, 1024]   out_w[:, shard].T      (in)
  ob    [128, 8]      out bias (or zeros)    (in)
  y_t   [1024, 2048]  partial y.T            (out)
  k_t   [512, 2048]   k cache (head-major)   (out)
  v_n   [2048, 512]   v cache (token-major)  (out)

Pipeline: QK projection -> kT/qT (transposed, bf16 copies for attention,
f32 for the k output); V projection -> natural layout (f32 out + bf16
ones-augmented copy for the attention w@v stationary); per head:
scores^T = kT.T@qT per 128-key block (fp32 psum), exp on ACT
(scale=1/8), causal mask on the diagonal block, w@v accumulation with a
ones column producing the softmax denominator for free; normalize via
reciprocal + gpsimd partition-broadcast; out projection in f32r.
"""

import numpy as np

B, N, D = 4, 2048, 1024
H, DH = 16, 64
NCORES = 8

_cache = {}


def _build():
    import concourse.bass as bass  # noqa: F401
    import concourse.mybir as mybir
    import concourse.tile as tile
    from concourse import bacc

    f32 = mybir.dt.float32
    f32r = mybir.dt.float32r
    bf16 = mybir.dt.bfloat16
    EXP = mybir.ActivationFunctionType.Exp
    COPY = mybir.ActivationFunctionType.Copy

    nc = bacc.Bacc("TRN2", target_bir_lowering=False, debug=False,
                   num_devices=NCORES)

    x_t = nc.dram_tensor("x_t", [1024, 2048], f32, kind="ExternalInput").ap()
    wqk_t = nc.dram_tensor("wqk_t", [1024, 1024], f32, kind="ExternalInput").ap()
    bqk = nc.dram_tensor("bqk", [128, 8], f32, kind="ExternalInput").ap()
    wv_t = nc.dram_tensor("wv_t", [1024, 512], f32, kind="ExternalInput").ap()
    bv = nc.dram_tensor("bv", [1, 512], f32, kind="ExternalInput").ap()
    ow_t = nc.dram_tensor("ow_t", [512, 1024], f32, kind="ExternalInput").ap()
    ob = nc.dram_tensor("ob", [128, 8], f32, kind="ExternalInput").ap()
    y_t = nc.dram_tensor("y_t", [1024, 2048], f32, kind="ExternalOutput").ap()
    k_t = nc.dram_tensor("k_t", [512, 2048], f32, kind="ExternalOutput").ap()
    v_n = nc.dram_tensor("v_n", [2048, 512], f32, kind="ExternalOutput").ap()

    import ml_dtypes
    mask_np = np.triu(np.ones((128, 128), dtype=np.float32)).astype(
        ml_dtypes.bfloat16)
    mask_dram = nc.inline_tensor(mask_np, name="causal_mask")

    with tile.TileContext(nc) as tc:
        from contextlib import ExitStack
        with ExitStack() as ctx:
            pers = ctx.enter_context(tc.tile_pool(name="pers", bufs=1))
            stage = ctx.enter_context(tc.tile_pool(name="stage", bufs=1))
            psum = ctx.enter_context(
                tc.tile_pool(name="psum", bufs=1, space="PSUM"))

            # ---- persistent loads -------------------------------------
            bqk_sb = pers.tile([128, 8], f32, tag="bqk_sb")
            nc.sync.dma_start(bqk_sb[:], bqk[:])
            ob_sb = pers.tile([128, 8], f32, tag="ob_sb")
            nc.sync.dma_start(ob_sb[:], ob[:])
            bv_sb = pers.tile([128, 512], f32, tag="bv_sb")
            nc.sync.dma_start(bv_sb[:], bv.to_broadcast((128, 512)))
            mask_sb = pers.tile([128, 128], bf16, tag="mask_sb")
            nc.sync.dma_start(mask_sb[:], mask_dram.ap())
            ow_sb = []
            for f in range(4):
                t = pers.tile([128, 1024], f32, tag=f"ow{f}", name=f"ow_sb{f}")
                nc.sync.dma_start(t[:], ow_t[128 * f:128 * (f + 1), :])
                ow_sb.append(t)

            qkT = []  # 8 x [128, 2048] bf16; 0-3 = qT tiles, 4-7 = kT tiles
            for m in range(8):
                t = pers.tile([128, 2048], bf16, tag=f"qkT{m}", name=f"qkT{m}")
                qkT.append(t)
            v_sb = []  # 16 x [128, 520] bf16, per-head 65-col groups (v | 1)
            for t_i in range(16):
                t = pers.tile([128, 520], bf16, tag=f"v{t_i}", name=f"v_sb{t_i}")
                v_sb.append(t)
            out2T = []  # 4 x [128, 2048] f32
            for f in range(4):
                t = pers.tile([128, 2048], f32, tag=f"o2_{f}", name=f"out2T{f}")
                out2T.append(t)

            # ---- phase 1+2: projections -------------------------------
            with tc.tile_pool(name="proj", bufs=1) as proj:
                xt = []
                for k in range(8):
                    t = proj.tile([128, 2048], f32, tag=f"xt{k}", name=f"xt{k}")
                    nc.sync.dma_start(t[:], x_t[128 * k:128 * (k + 1), :])
                    xt.append(t)
                wqk_sb = []
                for k in range(8):
                    t = proj.tile([128, 1024], f32, tag=f"wqk{k}",
                                  name=f"wqk_sb{k}")
                    nc.sync.dma_start(t[:], wqk_t[128 * k:128 * (k + 1), :])
                    wqk_sb.append(t)
                wv_sb = []
                for k in range(8):
                    t = proj.tile([128, 512], f32, tag=f"wv{k}",
                                  name=f"wv_sb{k}")
                    nc.sync.dma_start(t[:], wv_t[128 * k:128 * (k + 1), :])
                    wv_sb.append(t)

                # QK projection: out m-tile = 128 qk-features x 2048 tokens
                for m in range(8):
                    is_k = m >= 4
                    if is_k:
                        kst = stage.tile([128, 2048], f32, tag="kstage",
                                         bufs=2, name="kst")
                    for n in range(4):
                        ps = psum.tile([128, 512], f32, tag="mm", bufs=4,
                                       name="ps1")
                        for k in range(8):
                            nc.tensor.matmul(
                                ps[:],
                                wqk_sb[k][:, 128 * m:128 * (m + 1)].bitcast(f32r),
                                xt[k][:, 512 * n:512 * (n + 1)].bitcast(f32r),
                                start=(k == 0), stop=(k == 7))
                        nc.scalar.activation(
                            qkT[m][:, 512 * n:512 * (n + 1)], ps[:], COPY,
                            bias=bqk_sb[:, m:m + 1])
                        if is_k:
                            nc.vector.tensor_scalar_add(
                                kst[:, 512 * n:512 * (n + 1)], ps[:],
                                bqk_sb[:, m:m + 1])
                    if is_k:
                        nc.sync.dma_start(
                            k_t[128 * (m - 4):128 * (m - 3), :], kst[:])

                # V projection: out t-tile = 128 tokens x 512 v-features
                for t_i in range(16):
                    nc.vector.memset(v_sb[t_i][:], 1.0)
                    ps = psum.tile([128, 512], f32, tag="mm", bufs=4,
                                   name="ps2")
                    for k in range(8):
                        nc.tensor.matmul(
                            ps[:],
                            xt[k][:, 128 * t_i:128 * (t_i + 1)].bitcast(f32r),
                            wv_sb[k][:].bitcast(f32r),
                            start=(k == 0), stop=(k == 7))
                    vst = stage.tile([128, 512], f32, tag="vstage", bufs=3,
                                     name="vst")
                    nc.vector.tensor_add(vst[:], ps[:], bv_sb[:])
                    nc.sync.dma_start(v_n[128 * t_i:128 * (t_i + 1), :], vst[:])
                    nc.vector.tensor_copy(
                        v_sb[t_i].rearrange("p (h c) -> p h c", c=65)[:, :, 0:64],
                        vst.rearrange("p (h c) -> p h c", c=64))

            # ---- phase 3: attention, head by head ---------------------
            with tc.tile_pool(name="attn", bufs=1) as attn:
                for h in range(8):
                    mt, po = h // 2, (h % 2) * 64
                    qT_h = qkT[mt][po:po + 64, :]
                    kT_h = qkT[4 + mt][po:po + 64, :]
                    aug = psum.tile([65, 2048], f32, tag="aug", bufs=1,
                                    name="aug")
                    for j in range(16):
                        r, c0 = j % 4, j // 4
                        eT = attn.tile([128, 2048 - 512 * c0], bf16,
                                       tag="expT", bufs=3, name="eT")
                        for c in range(c0, 4):
                            ps = psum.tile([128, 512], f32, tag="mm", bufs=4,
                                           name="ps_s")
                            nc.tensor.matmul(
                                ps[:],
                                kT_h[:, 128 * j:128 * (j + 1)],
                                qT_h[:, 512 * c:512 * (c + 1)],
                                start=True, stop=True)
                            lo = 128 * r if c == c0 else 0
                            nc.scalar.activation(
                                eT[:, 512 * (c - c0) + lo:512 * (c - c0) + 512],
                                ps[:, lo:512], EXP, scale=0.125)
                        if r > 0:
                            nc.vector.memset(eT[:, 0:128 * r], 0.0)
                        nc.vector.tensor_mul(
                            eT[:, 128 * r:128 * (r + 1)],
                            eT[:, 128 * r:128 * (r + 1)], mask_sb[:])
                        vv = v_sb[j][:, 65 * h:65 * h + 65]
                        for c in range(c0, 4):
                            nc.tensor.matmul(
                                aug[:, 512 * c:512 * (c + 1)],
                                vv,
                                eT[:, 512 * (c - c0):512 * (c - c0) + 512],
                                start=(j == 0), stop=(j == 4 * c + 3))
                    # normalization
                    drow = attn.tile([1, 2048], f32, tag="drow", bufs=2,
                                     name="drow")
                    nc.sync.dma_start(drow[:], aug[64:65, :])
                    rrow = attn.tile([1, 2048], f32, tag="rrow", bufs=2,
                                     name="rrow")
                    nc.vector.reciprocal(rrow[:], drow[:])
                    bc = attn.tile([64, 2048], f32, tag="bcast", bufs=2,
                                   name="bc")
                    nc.gpsimd.partition_broadcast(bc[:], rrow[:])
                    if po == 0:
                        nc.vector.tensor_mul(
                            out2T[mt][0:64, :], aug[0:64, :], bc[:])
                    else:
                        mtmp = attn.tile([64, 2048], f32, tag="mtmp", bufs=2,
                                         name="mtmp")
                        nc.vector.tensor_mul(mtmp[:], aug[0:64, :], bc[:])
                        nc.sync.dma_start(out2T[mt][64:128, :], mtmp[:])

            # ---- phase 4: out projection ------------------------------
            for o in range(8):
                yst = stage.tile([128, 2048], f32, tag="ystage", bufs=2,
                                 name="yst")
                for n in range(4):
                    ps = psum.tile([128, 512], f32, tag="mm", bufs=4,
                                   name="ps4")
                    for f in range(4):
                        nc.tensor.matmul(
                            ps[:],
                            ow_sb[f][:, 128 * o:128 * (o + 1)].bitcast(f32r),
                            out2T[f][:, 512 * n:512 * (n + 1)].bitcast(f32r),
                            start=(f == 0), stop=(f == 3))
                    nc.scalar.activation(
                        yst[:, 512 * n:512 * (n + 1)], ps[:], COPY,
                        bias=ob_sb[:, o:o + 1])
                nc.sync.dma_start(y_t[128 * o:128 * (o + 1), :], yst[:])

    nc.compile()
    return nc


def _get_nc():
    if "nc" not in _cache:
        _cache["nc"] = _build()
    return _cache["nc"]


def _shard_inputs(X, W1_w, W1_b, out_w, out_b):
    X = np.asarray(X, dtype=np.float32)
    W1_w = np.asarray(W1_w, dtype=np.float32)
    W1_b = np.asarray(W1_b, dtype=np.float32)
    out_w = np.asarray(out_w, dtype=np.float32)
    out_b = np.asarray(out_b, dtype=np.float32)
    in_maps = []
    for c in range(NCORES):
        b, hg = c // 2, c % 2
        sl = slice(hg * 512, (hg + 1) * 512)
        wq = W1_w[0 * D:1 * D][sl]
        wk = W1_w[1 * D:2 * D][sl]
        wv = W1_w[2 * D:3 * D][sl]
        bq = W1_b[0 * D:1 * D][sl]
        bk = W1_b[1 * D:2 * D][sl]
        bvv = W1_b[2 * D:3 * D][sl]
        ob_full = out_b if hg == 0 else np.zeros_like(out_b)
        in_maps.append({
            "x_t": np.ascontiguousarray(X[b].T),
            "wqk_t": np.ascontiguousarray(np.concatenate([wq, wk], 0).T),
            "bqk": np.ascontiguousarray(
                np.concatenate([bq, bk]).reshape(8, 128).T),
            "wv_t": np.ascontiguousarray(wv.T),
            "bv": np.ascontiguousarray(bvv.reshape(1, 512)),
            "ow_t": np.ascontiguousarray(out_w[:, sl].T),
            "ob": np.ascontiguousarray(ob_full.reshape(8, 128).T),
        })
    return in_maps


def kernel(X, past_k, past_v, W1_w, W1_b, out_w, out_b):
    from concourse.bass_utils import run_bass_kernel_spmd

    nc = _get_nc()
    in_maps = _shard_inputs(X, W1_w, W1_b, out_w, out_b)
    res = run_bass_kernel_spmd(nc, in_maps, core_ids=list(range(NCORES)))
    y = np.empty((B, N, D), dtype=np.float32)
    k = np.empty((B, H, DH, N), dtype=np.float32)
    v = np.empty((B, H, N, DH), dtype=np.float32)
    for c in range(NCORES):
        b, hg = c // 2, c % 2
        r = res.results[c]
        hsl = slice(hg * 8, (hg + 1) * 8)
        k[b, hsl] = r["k_t"].reshape(8, DH, N)
        v[b, hsl] = r["v_n"].reshape(N, 8, DH).transpose(1, 0, 2)
        if hg == 0:
            y[b] = r["y_t"].T
        else:
            y[b] += r["y_t"].T
    return (y, k, v)


# revision 11
# speedup vs baseline: 1.1327x; 1.1327x over previous
"""Fused causal-attention block (QKV proj + attention + out proj) for
nn_Attn_49881750176227 on 8 TRN2 NeuronCores.

Sharding (hardcoded): core c -> batch b = c // 2, head-group hg = c % 2.
Each core handles 1 batch (2048 tokens) and 8 of the 16 heads, with the
matching column/row slices of W1 / out_proj (tensor parallel).  The
out-projection partial sums of the two cores sharing a batch are added
on the host during unsharding; k/v cache outputs shard cleanly by head.

Device layout (per core):
  x_t   [1024, 2048]  X[b].T                 (in, f32r)
  wqk_t [1024, 1024]  [Wq_shard; Wk_shard].T (in, f32r)
  bqk   [128, 8]      qk bias, tile-major    (in)
  wv_t  [1024, 512]   Wv_shard.T             (in, f32r)
  bv    [1, 512]      v bias row             (in)
  ow_t  [512, 1024]   out_w[:, shard].T      (in, f32r)
  ob    [128, 8]      out bias (or zeros)    (in)
  y_t   [1024, 2048]  partial y.T            (out)
  k_t   [512, 2048]   k cache (head-major)   (out)
  v_n   [2048, 512]   v cache (token-major)  (out)

Per head: scores^T = kT.T@qT per 128-key block (f32 psum, 1024-wide
tiles), exp on ACT (scale=1/8), causal mask on the diagonal block, w@v
accumulation with a ones column appended to V producing the softmax
denominator for free; normalize via reciprocal + gpsimd
partition-broadcast.  Odd heads' normalized output is moved from psum
partitions 0:64 to SBUF partitions 64:128 with a PE shift-matmul.
Dense projections run as f32r matmuls; attention matmuls in bf16.
"""

import numpy as np

B, N, D = 4, 2048, 1024
H, DH = 16, 64
NCORES = 8

_cache = {}


def _build():
    import os
    _PH = os.environ.get("KERNEL_PHASES", "1234")
    _NH = int(os.environ.get("KERNEL_HEADS", "8"))
    _DBG = os.environ.get("KERNEL_DBG", "") == "1"
    import concourse.bass as bass  # noqa: F401
    import concourse.mybir as mybir
    import concourse.tile as tile
    from concourse import bacc

    f32 = mybir.dt.float32
    f32r = mybir.dt.float32r
    bf16 = mybir.dt.bfloat16
    EXP = mybir.ActivationFunctionType.Exp
    IDENT = mybir.ActivationFunctionType.Identity

    nc = bacc.Bacc("TRN2", target_bir_lowering=False, debug=False,
                   num_devices=NCORES)

    x_t = nc.dram_tensor("x_t", [1024, 2048], f32r, kind="ExternalInput").ap()
    wqk_t = nc.dram_tensor("wqk_t", [1024, 1024], f32r, kind="ExternalInput").ap()
    bqk = nc.dram_tensor("bqk", [128, 8], f32, kind="ExternalInput").ap()
    wv_t = nc.dram_tensor("wv_t", [1024, 512], f32r, kind="ExternalInput").ap()
    bv = nc.dram_tensor("bv", [1, 512], f32, kind="ExternalInput").ap()
    ow_t = nc.dram_tensor("ow_t", [512, 1024], f32r, kind="ExternalInput").ap()
    ob = nc.dram_tensor("ob", [128, 8], f32, kind="ExternalInput").ap()
    y_t = nc.dram_tensor("y_t", [1024, 2048], f32, kind="ExternalOutput").ap()
    k_t = nc.dram_tensor("k_t", [512, 2048], f32, kind="ExternalOutput").ap()
    v_n = nc.dram_tensor("v_n", [2048, 512], f32, kind="ExternalOutput").ap()
    dbg = (nc.dram_tensor("dbg", [512, 2048], f32, kind="ExternalOutput").ap()
           if _DBG else None)

    import ml_dtypes
    mask_np = np.triu(np.ones((128, 128), dtype=np.float32)).astype(
        ml_dtypes.bfloat16)
    mask_dram = nc.inline_tensor(mask_np, name="causal_mask")
    # shift matrix: out[64+k] = in[k] for the odd-head partition move
    shift_np = np.zeros((64, 128), dtype=np.float32)
    shift_np[np.arange(64), 64 + np.arange(64)] = 1.0
    shift_dram = nc.inline_tensor(shift_np.astype(ml_dtypes.bfloat16),
                                  name="shift64")

    with tile.TileContext(nc) as tc:
        from contextlib import ExitStack
        with ExitStack() as ctx:
            pers = ctx.enter_context(tc.tile_pool(name="pers", bufs=1))
            psum = ctx.enter_context(
                tc.tile_pool(name="psum", bufs=1, space="PSUM"))

            # ---- persistent loads -------------------------------------
            bqk_sb = pers.tile([128, 8], f32, tag="bqk_sb")
            nc.sync.dma_start(bqk_sb[:], bqk[:])
            ob_sb = pers.tile([128, 8], f32, tag="ob_sb")
            nc.sync.dma_start(ob_sb[:], ob[:])
            bv_sb = pers.tile([128, 512], f32, tag="bv_sb")
            nc.sync.dma_start(bv_sb[:], bv.to_broadcast((128, 512)))
            mask_sb = pers.tile([128, 128], bf16, tag="mask_sb")
            nc.sync.dma_start(mask_sb[:], mask_dram.ap())
            shift_sb = pers.tile([64, 128], bf16, tag="shift_sb")
            nc.sync.dma_start(shift_sb[:], shift_dram.ap())

            qkT = []  # 8 x [128, 2048] bf16; 0-3 = qT tiles, 4-7 = kT tiles
            for m in range(8):
                t = pers.tile([128, 2048], bf16, tag=f"qkT{m}", name=f"qkT{m}")
                qkT.append(t)
            v_sb = []  # 16 x [128, 520] bf16, per-head 65-col groups (v | 1)
            for t_i in range(16):
                t = pers.tile([128, 520], bf16, tag=f"v{t_i}", name=f"v_sb{t_i}")
                v_sb.append(t)

            # ---- phase 1+2: projections -------------------------------
            with tc.tile_pool(name="proj", bufs=1) as proj:
                xt = []
                for k in range(8):
                    t = proj.tile([128, 2048], f32r, tag=f"xt{k}", name=f"xt{k}")
                    nc.sync.dma_start(t[:], x_t[128 * k:128 * (k + 1), :])
                    xt.append(t)
                wqk_sb = []
                for k in range(8):
                    t = proj.tile([128, 1024], f32r, tag=f"wqk{k}",
                                  name=f"wqk_sb{k}")
                    nc.sync.dma_start(t[:], wqk_t[128 * k:128 * (k + 1), :])
                    wqk_sb.append(t)
                wv_sb = []
                for k in range(8):
                    t = proj.tile([128, 512], f32r, tag=f"wv{k}",
                                  name=f"wv_sb{k}")
                    nc.sync.dma_start(t[:], wv_t[128 * k:128 * (k + 1), :])
                    wv_sb.append(t)

                # QK projection: out m-tile = 128 qk-features x 2048 tokens
                for m in (range(8) if "1" in _PH else []):
                    is_k = m >= 4
                    if is_k:
                        kst = proj.tile([128, 2048], f32, tag="kstage",
                                        bufs=2, name="kst")
                    for n2 in range(2):  # 1024-wide halves
                        ps = psum.tile([128, 1024], f32, tag="sc", bufs=2,
                                       name="ps1")
                        for half in range(2):
                            n = 2 * n2 + half
                            for k in range(8):
                                nc.tensor.matmul(
                                    ps[:, 512 * half:512 * (half + 1)],
                                    wqk_sb[k][:, 128 * m:128 * (m + 1)],
                                    xt[k][:, 512 * n:512 * (n + 1)],
                                    start=(k == 0), stop=(k == 7))
                        nc.scalar.activation(
                            qkT[m][:, 1024 * n2:1024 * (n2 + 1)], ps[:],
                            IDENT, bias=bqk_sb[:, m:m + 1])
                        if is_k:
                            nc.vector.tensor_scalar_add(
                                kst[:, 1024 * n2:1024 * (n2 + 1)], ps[:],
                                bqk_sb[:, m:m + 1])
                    if is_k:
                        nc.sync.dma_start(
                            k_t[128 * (m - 4):128 * (m - 3), :], kst[:])

                # V projection: out t-tile = 128 tokens x 512 v-features
                for t_i in (range(16) if "2" in _PH else []):
                    nc.vector.memset(v_sb[t_i][:], 1.0)
                    ps = psum.tile([128, 512], f32, tag="sc", bufs=2,
                                   name="ps2")
                    for k in range(8):
                        nc.tensor.matmul(
                            ps[:],
                            xt[k][:, 128 * t_i:128 * (t_i + 1)],
                            wv_sb[k][:],
                            start=(k == 0), stop=(k == 7))
                    vst = proj.tile([128, 512], f32, tag="vstage", bufs=3,
                                    name="vst")
                    nc.vector.tensor_add(vst[:], ps[:], bv_sb[:])
                    nc.sync.dma_start(v_n[128 * t_i:128 * (t_i + 1), :], vst[:])
                    nc.vector.tensor_copy(
                        v_sb[t_i].rearrange("p (h c) -> p h c", c=65)[:, :, 0:64],
                        vst.rearrange("p (h c) -> p h c", c=64))

            # ---- late pool: out-proj weights + outputs ----------------
            with ExitStack() as lctx:
                late = lctx.enter_context(tc.tile_pool(name="late", bufs=1))
                ow_sb = []
                for f in range(4):
                    t = late.tile([128, 1024], f32r, tag=f"ow{f}",
                                  name=f"ow_sb{f}")
                    nc.sync.dma_start(t[:], ow_t[128 * f:128 * (f + 1), :])
                    ow_sb.append(t)
                out2T = []  # 4 x [128, 2048] f32r
                for f in range(4):
                    t = late.tile([128, 2048], f32r, tag=f"o2_{f}",
                                  name=f"out2T{f}")
                    out2T.append(t)

                # ---- phase 3: attention, head by head -----------------
                with tc.tile_pool(name="attn", bufs=1) as attn:
                    for h in (range(_NH) if "3" in _PH else []):
                        mt, po = h // 2, (h % 2) * 64
                        qT_h = qkT[mt][po:po + 64, :]
                        kT_h = qkT[4 + mt][po:po + 64, :]
                        aug = psum.tile([65, 2048], f32, tag="aug", bufs=1,
                                        name="aug")
                        for j in range(16):
                            r, c0 = j % 4, j // 4
                            qa = 512 * c0
                            eT = attn.tile([128, 2048 - qa], bf16,
                                           tag="expT", bufs=3, name="eT")
                            for w in range(j // 8, 2):
                                sc = psum.tile([128, 1024], f32, tag="sc",
                                               bufs=2, name="sc")
                                for c in range(max(c0, 2 * w), 2 * w + 2):
                                    nc.tensor.matmul(
                                        sc[:, 512 * c - 1024 * w:
                                           512 * (c + 1) - 1024 * w],
                                        kT_h[:, 128 * j:128 * (j + 1)],
                                        qT_h[:, 512 * c:512 * (c + 1)],
                                        start=True, stop=True)
                                lo_q = max(128 * j, 1024 * w)
                                nc.scalar.activation(
                                    eT[:, lo_q - qa:1024 * (w + 1) - qa],
                                    sc[:, lo_q - 1024 * w:1024],
                                    EXP, scale=0.125)
                            if r > 0:
                                nc.vector.memset(eT[:, 0:128 * r], 0.0)
                            nc.vector.tensor_mul(
                                eT[:, 128 * r:128 * (r + 1)],
                                eT[:, 128 * r:128 * (r + 1)], mask_sb[:])
                            vv = v_sb[j][:, 65 * h:65 * h + 65]
                            for c in range(c0, 4):
                                nc.tensor.matmul(
                                    aug[:, 512 * c:512 * (c + 1)],
                                    vv,
                                    eT[:, 512 * c - qa:512 * (c + 1) - qa],
                                    start=(j == 0), stop=(j == 4 * c + 3))
                        # normalization: psum row 64 -> sbuf (aligned), then
                        # SBUF->SBUF DMA down to partition 0.
                        s64 = attn.tile([65, 2048], f32, tag="s64", bufs=1,
                                        name="s64")
                        nc.vector.tensor_copy(s64[64:65, :], aug[64:65, :])
                        drow = attn.tile([1, 2048], f32, tag="drow", bufs=1,
                                         name="drow")
                        nc.sync.dma_start(drow[:], s64[64:65, :])
                        rrow = attn.tile([1, 2048], f32, tag="rrow", bufs=1,
                                         name="rrow")
                        nc.vector.reciprocal(rrow[:], drow[:])
                        bc = attn.tile([64, 2048], f32, tag="bcast", bufs=1,
                                       name="bc")
                        nc.gpsimd.partition_broadcast(bc[:], rrow[:])
                        if po == 0:
                            nc.vector.tensor_mul(
                                out2T[mt][0:64, :], aug[0:64, :], bc[:])
                        else:
                            # normalize into bf16, then PE-shift partitions
                            # 0:64 -> 64:128 and evict to out2T rows 64:128.
                            mtmp = attn.tile([64, 2048], bf16, tag="mtmp",
                                             bufs=1, name="mtmp")
                            nc.vector.tensor_mul(mtmp[:], aug[0:64, :], bc[:])
                            for w in range(2):
                                sc = psum.tile([128, 1024], f32, tag="sc",
                                               bufs=2, name="shift_ps")
                                nc.tensor.matmul(
                                    sc[:], shift_sb[:],
                                    mtmp[:, 1024 * w:1024 * (w + 1)],
                                    start=True, stop=True)
                                nc.vector.tensor_copy(
                                    out2T[mt][64:128,
                                              1024 * w:1024 * (w + 1)],
                                    sc[64:128, :])

                if _DBG:
                    for f in range(4):
                        nc.sync.dma_start(dbg[128 * f:128 * (f + 1), :],
                                          out2T[f][:].bitcast(f32))

                # ---- phase 4: out projection --------------------------
                for o in (range(8) if "4" in _PH else []):
                    yst = late.tile([128, 2048], f32, tag="ystage", bufs=2,
                                    name="yst")
                    for n2 in range(2):
                        ps = psum.tile([128, 1024], f32, tag="sc", bufs=2,
                                       name="ps4")
                        for half in range(2):
                            n = 2 * n2 + half
                            for f in range(4):
                                nc.tensor.matmul(
                                    ps[:, 512 * half:512 * (half + 1)],
                                    ow_sb[f][:, 128 * o:128 * (o + 1)],
                                    out2T[f][:, 512 * n:512 * (n + 1)],
                                    start=(f == 0), stop=(f == 3))
                        nc.scalar.activation(
                            yst[:, 1024 * n2:1024 * (n2 + 1)], ps[:],
                            IDENT, bias=ob_sb[:, o:o + 1])
                    nc.sync.dma_start(y_t[128 * o:128 * (o + 1), :], yst[:])

    nc.compile()
    return nc


def _get_nc():
    if "nc" not in _cache:
        _cache["nc"] = _build()
    return _cache["nc"]


def _shard_inputs(X, W1_w, W1_b, out_w, out_b):
    X = np.asarray(X, dtype=np.float32)
    W1_w = np.asarray(W1_w, dtype=np.float32)
    W1_b = np.asarray(W1_b, dtype=np.float32)
    out_w = np.asarray(out_w, dtype=np.float32)
    out_b = np.asarray(out_b, dtype=np.float32)
    in_maps = []
    for c in range(NCORES):
        b, hg = c // 2, c % 2
        sl = slice(hg * 512, (hg + 1) * 512)
        wq = W1_w[0 * D:1 * D][sl]
        wk = W1_w[1 * D:2 * D][sl]
        wv = W1_w[2 * D:3 * D][sl]
        bq = W1_b[0 * D:1 * D][sl]
        bk = W1_b[1 * D:2 * D][sl]
        bvv = W1_b[2 * D:3 * D][sl]
        ob_full = out_b if hg == 0 else np.zeros_like(out_b)
        in_maps.append({
            "x_t": np.ascontiguousarray(X[b].T),
            "wqk_t": np.ascontiguousarray(np.concatenate([wq, wk], 0).T),
            "bqk": np.ascontiguousarray(
                np.concatenate([bq, bk]).reshape(8, 128).T),
            "wv_t": np.ascontiguousarray(wv.T),
            "bv": np.ascontiguousarray(bvv.reshape(1, 512)),
            "ow_t": np.ascontiguousarray(out_w[:, sl].T),
            "ob": np.ascontiguousarray(ob_full.reshape(8, 128).T),
        })
    return in_maps


def kernel(X, past_k, past_v, W1_w, W1_b, out_w, out_b):
    from concourse.bass_utils import run_bass_kernel_spmd

    nc = _get_nc()
    in_maps = _shard_inputs(X, W1_w, W1_b, out_w, out_b)
    res = run_bass_kernel_spmd(nc, in_maps, core_ids=list(range(NCORES)))
    _cache["last_res"] = res
    y = np.empty((B, N, D), dtype=np.float32)
    k = np.empty((B, H, DH, N), dtype=np.float32)
    v = np.empty((B, H, N, DH), dtype=np.float32)
    for c in range(NCORES):
        b, hg = c // 2, c % 2
        r = res.results[c]
        hsl = slice(hg * 8, (hg + 1) * 8)
        k[b, hsl] = r["k_t"].reshape(8, DH, N)
        v[b, hsl] = r["v_n"].reshape(N, 8, DH).transpose(1, 0, 2)
        if hg == 0:
            y[b] = r["y_t"].T
        else:
            y[b] += r["y_t"].T
    return (y, k, v)
